# revision 45
# baseline (speedup 1.0000x reference)
"""Trainium2 Bass kernel for the AxialShift block (4x96x256x256, fp32).

Self-contained: builds an 8-core SPMD Bass program, compiles it once,
and runs it via run_bass_kernel_spmd.

Sharding: each core runs S=2 streams; stream s of core k handles a
quarter-sample slab (64 rows) of sample 2s + k//4.

Pipeline per core (x read once as fp16, out written once):
  phase A : conv1 as N=1024 fp16 matmuls over the 66-row frame (64 true
            rows + halo); PSUM evacuated fp16 into a zero-padded
            [96,66,258] frame on ScalarE; per-channel bn_stats on DVE.
  AR1     : ONE 16-byte AllReduce carrying both streams' GroupNorm-1
            partials (groups of 4 cores sharing a sample).
  GN+B    : GN1 scale/bias + erf-Gelu in place (ScalarE; rsqrt computed
            on DVE via bit-trick+Newton so no Sqrt ACT-table load); the
            4 axial-shift branch convs as 12 chunk-masked K=97 N=1024
            matmuls per 4-row supertile, grouped in branch pairs into
            [M,2,1024] PSUM tiles; one fused Gelu ACTIVATE per pair;
            branch sums split DVE/GpSimd; bn_stats for GroupNorm-2.
  AR2_s   : per-stream 8-byte AllReduce; conv3 (weights pre-folded with
            gamma2 on host, unscaled by std) runs immediately after each
            stream's phase B, writing fp16 results in place over the
            branch-sum buffer. Post-AR2 the per-sample 1/std2 scale and
            bias are applied on DVE/ScalarE and tiles stream out, so
            stream 0's stores overlap stream 1's compute.
"""
import sys

sys.path.insert(0, "/opt/trn_rl_repo")

import numpy as np

import concourse.bass as bass
import concourse.bacc as bacc
import concourse.tile as tile
from concourse import mybir

F32 = mybir.dt.float32
F16 = mybir.dt.float16
I32 = mybir.dt.int32

C = 96
M = 128           # matmul output width (96 channels + 32 zero pad -> FWL)
H = 256
W = 256
B = 4
WP = W + 2
N_CORES = 8
S = 2             # streams per core
ROWS_SLAB = H * B // (N_CORES * S)       # 64
EPS = 1e-5
AF = mybir.ActivationFunctionType
ALU = mybir.AluOpType
MAGIC = 0x5F3759DF

# (dh, dw) read offsets per chunk j=0,1,2 (s_j = -1, 0, +1):
BR_LR = [(0, 1), (0, 0), (0, -1)]
BR_LDIAG = [(1, 1), (0, 0), (-1, -1)]
BR_TD = [(1, 0), (0, 0), (-1, 0)]
BR_RDIAG = [(1, -1), (0, 0), (-1, 1)]
PAIRS = [(0, [BR_LR, BR_LDIAG]), (3, [BR_TD, BR_RDIAG])]


def _bcast(ap, nparts):
    return bass.AP(tensor=ap.tensor, offset=ap.offset,
                   ap=[[0, nparts]] + list(ap.ap[1:]))


def _rsqrt_cols(eng, con, ve, name):
    """rsqrt(ve) for a [C, ncol] fp32 tile on DVE or GpSimd (no ACT table).

    Quake initial guess + 3 Newton iterations (rel err ~4e-6)."""
    ncol = ve.shape[1]
    y = con.tile([C, ncol], F32, name=f"{name}_y")
    t = con.tile([C, ncol], F32, name=f"{name}_t")
    eng.tensor_scalar(out=t[:].bitcast(I32), in0=ve[:].bitcast(I32),
                      scalar1=1, scalar2=None,
                      op0=ALU.logical_shift_right)
    eng.tensor_scalar(out=y[:].bitcast(I32), in0=t[:].bitcast(I32),
                      scalar1=-1, scalar2=MAGIC,
                      op0=ALU.mult, op1=ALU.add)
    for _ in range(3):
        eng.tensor_mul(out=t[:], in0=y[:], in1=y[:])
        eng.tensor_mul(out=t[:], in0=t[:], in1=ve[:])
        eng.tensor_scalar(out=t[:], in0=t[:], scalar1=-0.5,
                          scalar2=1.5, op0=ALU.mult, op1=ALU.add)
        eng.tensor_mul(out=y[:], in0=y[:], in1=t[:])
    return y


class _Stream:
    """Per-stream state; stages are emitted by the orchestrator."""

    def __init__(self, nc, tc, pools, RH, groups, io, s):
        self.nc, self.tc, self.s = nc, tc, s
        self.p = pools
        self.RH = RH                      # 64 true rows
        self.RF = RH + 2                  # 66 frame rows
        self.NB = (self.RF + 3) // 4      # 17 phase-A batches (4 rows)
        self.NST = RH // 4                # 16 supertiles (4 rows x 256)
        self.groups = groups
        self.inv_n = 1.0 / (len(groups[0]) * C)
        self.io = io
        con = pools["consts"]
        big = pools["big"]
        self.xact = big.tile([C + 1, self.RF, WP], F16, name=f"xact{s}")
        self.opre = big.tile([C, self.NST, 4 * W], F16, name=f"opre{s}")
        self.stats1 = con.tile([C, 2 * self.NB - 1, 6], F32, name=f"st1_{s}")
        self.stats2 = con.tile([C, 2 * self.NST, 6], F32, name=f"st2_{s}")
        dram = pools["dram"]
        self.d1i = dram.tile([1, 2], F32, name=f"d1i{s}")
        self.d1o = dram.tile([1, 2], F32, name=f"d1o{s}")
        self.d2i = dram.tile([1, 2], F32, name=f"d2i{s}")
        self.d2o = dram.tile([1, 2], F32, name=f"d2o{s}")

    # ---------------- phase A ----------------
    def stage_a_init(self):
        nc = self.nc
        nc.vector.memset(self.xact[0:C, :, 0:1], 0.0)
        nc.vector.memset(self.xact[0:C, :, WP - 1:WP], 0.0)
        onesrow = self.io["onesrow"][:]
        nc.gpsimd.dma_start(
            out=self.xact[C:C + 1, :, :],
            in_=bass.AP(tensor=onesrow.tensor, offset=onesrow.offset,
                        ap=[[0, 1], [0, self.RF]] + list(onesrow.ap[1:])))
        self._xt, self._xt_base = None, 0

    def stage_a_batch(self, psa, b0, evac_dve=False):
        nc, s = self.nc, self.s
        xin = self.p["xin"]
        xs = self.io["xs"][s]
        r0 = 4 * b0
        nr = min(4, self.RF - r0)
        if r0 % 8 == 0:
            self._xt = xin.tile([C, 8, W], F16, tag="xt")
            nrows = min(8, self.RF - r0)
            nc.sync.dma_start(out=self._xt[:, 0:nrows, :],
                              in_=xs[:, r0:r0 + nrows, :])
            self._xt_base = r0
        xt = self._xt
        rr = r0 - self._xt_base
        pt = psa.tile([M, 4, W], F32, tag="pta")
        for h in range(0, nr, 2):
            nc.tensor.matmul(
                out=pt[:, h:h + 2, :].rearrange("p r w -> p (r w)"),
                lhsT=self.p["w1t"][:],
                rhs=xt[:, rr + h:rr + h + 2, :],
                start=True, stop=True)
        if evac_dve:
            nc.vector.tensor_copy(out=self.xact[0:C, r0:r0 + nr, 1:W + 1],
                                  in_=pt[0:C, 0:nr, :])
        else:
            nc.scalar.copy(out=self.xact[0:C, r0:r0 + nr, 1:W + 1],
                           in_=pt[0:C, 0:nr, :])
        if b0 == 0:
            nc.vector.bn_stats(out=self.stats1[:, 0, :],
                               in_=pt[0:C, 1:3, :].rearrange(
                                   "p r w -> p (r w)"))
            nc.vector.bn_stats(out=self.stats1[:, 1, :], in_=pt[0:C, 3, :])
        elif r0 + nr >= self.RF:
            nc.vector.bn_stats(out=self.stats1[:, 2 * b0, :],
                               in_=pt[0:C, 0, :])
        else:
            for h in range(2):
                nc.vector.bn_stats(
                    out=self.stats1[:, 2 * b0 + h, :],
                    in_=pt[0:C, 2 * h:2 * h + 2, :].rearrange(
                        "p r w -> p (r w)"))

    def stage_a_finish(self, psa):
        nc, s = self.nc, self.s
        con = self.p["consts"]
        cols = self.p["cols"]
        mv1 = con.tile([C, 2], F32, name=f"mv1_{s}")
        nc.vector.bn_aggr(out=mv1[:], in_=self.stats1[:])
        pack1 = con.tile([C, 2], F32, name=f"pk1_{s}")
        nc.vector.tensor_add(out=pack1[:, 0:1], in0=mv1[:, 0:1],
                             in1=cols[:, 0:1])
        tsq = con.tile([C, 1], F32, name=f"t1q_{s}")
        nc.vector.tensor_mul(out=tsq[:], in0=pack1[:, 0:1], in1=pack1[:, 0:1])
        nc.vector.tensor_add(out=pack1[:, 1:2], in0=mv1[:, 1:2], in1=tsq[:])
        spt = psa.tile([M, 4, W], F32, tag="pta")
        nc.tensor.matmul(out=spt[0:1, 0, 0:2], lhsT=self.p["ones96"][:],
                         rhs=pack1[:], start=True, stop=True)
        ar_in = con.tile([1, 2], F32, name=f"ar1i_{s}")
        nc.scalar.copy(out=ar_in[:], in_=spt[0:1, 0, 0:2])
        nc.sync.dma_start(out=self.d1i[:], in_=ar_in[:])
        nc.gpsimd.collective_compute(
            "AllReduce", ALU.add, replica_groups=self.groups,
            ins=[self.d1i.opt()], outs=[self.d1o.opt()])

    # ---------------- GN1 scalars (DVE; emit where the queue has slack) --
    def post_ar1(self):
        nc, s = self.nc, self.s
        eng = nc.vector
        con = self.p["consts"]
        cols = self.p["cols"]
        ar1 = con.tile([C, 2], F32, name=f"ar1_{s}")
        nc.gpsimd.dma_start(out=ar1[:], in_=_bcast(self.d1o[:], C))
        mu = con.tile([C, 1], F32, name=f"mu1_{s}")
        eng.tensor_scalar_mul(out=mu[:], in0=ar1[:, 0:1],
                              scalar1=self.inv_n)
        ve = con.tile([C, 1], F32, name=f"v1_{s}")
        eng.tensor_scalar(out=ve[:], in0=ar1[:, 1:2],
                          scalar1=self.inv_n, scalar2=EPS,
                          op0=ALU.mult, op1=ALU.add)
        musq = con.tile([C, 1], F32, name=f"mq1_{s}")
        eng.tensor_mul(out=musq[:], in0=mu[:], in1=mu[:])
        eng.tensor_sub(out=ve[:], in0=ve[:], in1=musq[:])
        inv = _rsqrt_cols(eng, con, ve, f"rs1_{s}")
        self.scale1 = con.tile([C, 1], F32, name=f"sc1_{s}")
        eng.tensor_mul(out=self.scale1[:], in0=inv[:], in1=cols[:, 1:2])
        self.bias1 = con.tile([C, 1], F32, name=f"bi1_{s}")
        eng.tensor_sub(out=self.bias1[:], in0=cols[:, 0:1], in1=mu[:])
        eng.tensor_mul(out=self.bias1[:], in0=self.bias1[:],
                       in1=self.scale1[:])
        eng.tensor_add(out=self.bias1[:], in0=self.bias1[:],
                       in1=cols[:, 2:3])

    def _gn_chunk(self, r0, r1):
        nc = self.nc
        nc.scalar.activation(out=self.xact[0:C, r0:r1, 1:W + 1],
                             in_=self.xact[0:C, r0:r1, 1:W + 1],
                             func=AF.Gelu, bias=self.bias1[:],
                             scale=self.scale1[:])
        if r0 == 0:
            nc.vector.tensor_scalar_mul(out=self.xact[0:C, 0:1, :],
                                        in0=self.xact[0:C, 0:1, :],
                                        scalar1=self.p["em"][:, 2 * self.s:
                                                            2 * self.s + 1])
        if r1 == self.RF:
            nc.vector.tensor_scalar_mul(
                out=self.xact[0:C, self.RF - 1:self.RF, :],
                in0=self.xact[0:C, self.RF - 1:self.RF, :],
                scalar1=self.p["em"][:, 2 * self.s + 1:2 * self.s + 2])

    # ---------------- GN-apply + phase B ----------------
    def stage_b_start(self):
        self._gn_r = 0

    def stage_b_tile(self, psb, st):
        nc, s = self.nc, self.s
        wbm = self.p["wbm"]
        gst, tmp = self.p["gst"], self.p["tmp"]
        need = min(4 * st + 6, self.RF)
        while self._gn_r < need:
            r1 = min(self._gn_r + 8, self.RF)
            self._gn_chunk(self._gn_r, r1)
            self._gn_r = r1
        pr = 4 * st + 1
        adds = []
        for pi, (wsel, branches) in enumerate(PAIRS):
            pt = psb.tile([M, 2, 4 * W], F32, tag="pair")
            for b, ds in enumerate(branches):
                for h in range(2):
                    for j, (dh, dw) in enumerate(ds):
                        bi = wsel + j
                        nc.tensor.matmul(
                            out=pt[:, b, 512 * h:512 * (h + 1)],
                            lhsT=wbm[:, bi * M:(bi + 1) * M],
                            rhs=self.xact[0:C + 1,
                                          pr + dh + 2 * h:pr + dh + 2 * h + 2,
                                          1 + dw:1 + dw + W],
                            start=(j == 0), stop=(j == 2))
            g = gst.tile([C, 2, 4 * W], F16, tag="g")
            nc.scalar.activation(out=g[:], in_=pt[0:C, :, :],
                                 func=AF.Gelu, bias=self.p["zcol"][:])
            a = tmp.tile([C, 4 * W], F16, tag=f"a{pi}")
            eng = nc.vector if pi == 0 else nc.gpsimd
            eng.tensor_add(out=a[:], in0=g[:, 0, :], in1=g[:, 1, :])
            adds.append(a)
        od = self.opre[:, st, :]
        nc.vector.tensor_add(out=od, in0=adds[0][:], in1=adds[1][:])
        for h in range(2):
            nc.vector.bn_stats(out=self.stats2[:, 2 * st + h, :],
                               in_=self.opre[:, st, 512 * h:512 * (h + 1)])

    def stage_b_finish(self, psb):
        nc, s = self.nc, self.s
        con = self.p["consts"]
        mv2 = con.tile([C, 2], F32, name=f"mv2_{s}")
        nc.vector.bn_aggr(out=mv2[:], in_=self.stats2[:])
        pack2 = con.tile([C, 2], F32, name=f"pk2_{s}")
        nc.vector.tensor_copy(out=pack2[:, 0:1], in_=mv2[:, 0:1])
        tsq = con.tile([C, 1], F32, name=f"t2q_{s}")
        nc.vector.tensor_mul(out=tsq[:], in0=mv2[:, 0:1], in1=mv2[:, 0:1])
        nc.vector.tensor_add(out=pack2[:, 1:2], in0=mv2[:, 1:2], in1=tsq[:])
        spt = psb.tile([M, 2, 4 * W], F32, tag="pair")
        nc.tensor.matmul(out=spt[0:1, 0, 0:2], lhsT=self.p["ones96"][:],
                         rhs=pack2[:], start=True, stop=True)
        ar_in = con.tile([1, 2], F32, name=f"ar2i_{s}")
        nc.scalar.copy(out=ar_in[:], in_=spt[0:1, 0, 0:2])
        nc.sync.dma_start(out=self.d2i[:], in_=ar_in[:])
        nc.gpsimd.collective_compute(
            "AllReduce", ALU.add, replica_groups=self.groups,
            ins=[self.d2i.opt()], outs=[self.d2o.opt()])

    # ---------------- conv3 (pre-AR2 scale), in place ----------------
    def stage_c1_pair(self, psb, t2):
        nc = self.nc
        w3g = self.p["w3g16"]
        pc = psb.tile([M, 2, 4 * W], F32, tag="pair")
        for j in range(2):
            t = 2 * t2 + j
            for h in range(2):
                nc.tensor.matmul(out=pc[:, j, 512 * h:512 * (h + 1)],
                                 lhsT=w3g[:],
                                 rhs=self.opre[:, t,
                                               512 * h:512 * (h + 1)],
                                 start=True, stop=True)
        # split the PSUM evacuation across engines so the pair-pool slot
        # frees quickly (a single 2048-elem DVE cast is a 2.3us hold)
        nc.scalar.copy(out=self.opre[:, 2 * t2, :], in_=pc[0:C, 0, :])
        nc.vector.tensor_copy(out=self.opre[:, 2 * t2 + 1, :],
                              in_=pc[0:C, 1, :])

    # ---------------- GN2 scalars (all DVE) ----------------
    def post_ar2(self):
        nc, s = self.nc, self.s
        con = self.p["consts"]
        cols = self.p["cols"]
        ar2 = con.tile([C, 2], F32, name=f"ar2_{s}")
        nc.sync.dma_start(out=ar2[:], in_=_bcast(self.d2o[:], C))
        mu = con.tile([C, 1], F32, name=f"mu2_{s}")
        nc.vector.tensor_scalar_mul(out=mu[:], in0=ar2[:, 0:1],
                                    scalar1=self.inv_n)
        ve = con.tile([C, 1], F32, name=f"v2_{s}")
        nc.vector.tensor_scalar(out=ve[:], in0=ar2[:, 1:2],
                                scalar1=self.inv_n, scalar2=EPS,
                                op0=ALU.mult, op1=ALU.add)
        musq = con.tile([C, 1], F32, name=f"mq2_{s}")
        nc.vector.tensor_mul(out=musq[:], in0=mu[:], in1=mu[:])
        nc.vector.tensor_sub(out=ve[:], in0=ve[:], in1=musq[:])
        self.inv2 = _rsqrt_cols(nc.vector, con, ve, f"rs2_{s}")
        self.ccol = con.tile([C, 1], F32, name=f"cc_{s}")
        nc.vector.tensor_mul(out=self.ccol[:], in0=self.inv2[:], in1=mu[:])
        nc.vector.tensor_mul(out=self.ccol[:], in0=self.ccol[:],
                             in1=cols[:, 6:7])
        nc.vector.tensor_sub(out=self.ccol[:], in0=cols[:, 5:6],
                             in1=self.ccol[:])

    # ---------------- post-AR2 scale + store ----------------
    def stage_c2_slice(self, t, on_scalar=False):
        nc = self.nc
        ost = self.p["ost"]
        out = self.io["out"][self.s]
        o = ost.tile([C, 4 * W], F32, tag="o")
        if on_scalar:
            nc.scalar.activation(out=o[:], in_=self.opre[:, t, :],
                                 func=AF.Identity, bias=self.ccol[:],
                                 scale=self.inv2[:])
        else:
            nc.vector.tensor_scalar(out=o[:], in0=self.opre[:, t, :],
                                    scalar1=self.inv2[:],
                                    scalar2=self.ccol[:],
                                    op0=ALU.mult, op1=ALU.add)
        nc.sync.dma_start(
            out=out[:, 4 * t:4 * t + 4, :],
            in_=o[:].rearrange("p (r w) -> p r w", w=W))

    def stage_c2_pair(self, t2, on_scalar=False):
        """Tail variant: scale+store two contiguous slices in one op and
        one store DMA (halves instruction overhead in the tail)."""
        nc = self.nc
        ost = self.p["ost"]
        out = self.io["out"][self.s]
        o = ost.tile([C, 2, 4 * W], F32, tag="o2")
        src = self.opre[:, 2 * t2:2 * t2 + 2, :]
        if on_scalar:
            nc.scalar.activation(out=o[:], in_=src,
                                 func=AF.Identity, bias=self.ccol[:],
                                 scale=self.inv2[:])
        else:
            nc.vector.tensor_scalar(out=o[:], in0=src,
                                    scalar1=self.inv2[:],
                                    scalar2=self.ccol[:],
                                    op0=ALU.mult, op1=ALU.add)
        nc.sync.dma_start(
            out=out[:, 8 * t2:8 * t2 + 8, :],
            in_=o[:].rearrange("p n (r w) -> p (n r) w", w=W))


def _emit(nc, tc, ctx, RH, groups, io):
    pools = {
        "consts": ctx.enter_context(tc.tile_pool(name="consts", bufs=1)),
        "big": ctx.enter_context(tc.tile_pool(name="big", bufs=1)),
        "xin": ctx.enter_context(tc.tile_pool(name="xin", bufs=3)),
        "gst": ctx.enter_context(tc.tile_pool(name="gst", bufs=3)),
        "tmp": ctx.enter_context(tc.tile_pool(name="tmp", bufs=4)),
        "ost": ctx.enter_context(tc.tile_pool(name="ost", bufs=2)),
        "dram": ctx.enter_context(tc.tile_pool(name="dram", bufs=1,
                                               space="DRAM")),
    }
    con = pools["consts"]
    # consts go on the vector/scalar DMA queues so the sync queue starts
    # with the first xs input loads (phase A ramps ~5us earlier)
    w1t = con.tile([C, M], F16)
    nc.scalar.dma_start(out=w1t[:], in_=io["w1t"][:])
    wbm = con.tile([C + 1, 6 * M], F16)
    nc.scalar.dma_start(out=wbm[:], in_=io["wbm"][:])
    w3g16 = con.tile([C, M], F16)
    nc.scalar.dma_start(out=w3g16[:], in_=io["w3g16"][:])
    cols = con.tile([C, 7], F32)
    nc.scalar.dma_start(out=cols[:], in_=io["cols"][:])
    em = con.tile([C, 2 * S], F32)
    nc.gpsimd.dma_start(out=em[:], in_=_bcast(io["em"][:], C))
    ones96 = con.tile([C, 1], F32)
    nc.vector.memset(ones96[:], 1.0)
    zcol = con.tile([C, 1], F32)
    nc.vector.memset(zcol[:], 0.0)
    # preload the Gelu ACT table so the first GN chunk (on the critical
    # path right after AR1) doesn't pay the ~2.6us ACT_TABLE_LOAD
    gwarm = con.tile([C, 1], F32)
    nc.scalar.activation(out=gwarm[:], in_=zcol[:], func=AF.Gelu,
                         bias=zcol[:])
    pools.update(w1t=w1t, wbm=wbm, w3g16=w3g16, cols=cols, em=em,
                 ones96=ones96, zcol=zcol)

    dram = pools["dram"]
    # warm up the collectives firmware path (result unused)
    dw_i = dram.tile([1, 2], F32)
    dw_o = dram.tile([1, 2], F32)
    warm = con.tile([1, 2], F32)
    nc.vector.memset(warm[:], 0.0)
    nc.sync.dma_start(out=dw_i[:], in_=warm[:])
    nc.gpsimd.collective_compute("AllReduce", ALU.add, replica_groups=groups,
                                 ins=[dw_i.opt()], outs=[dw_o.opt()])

    streams = [_Stream(nc, tc, pools, RH, groups, io, s) for s in range(S)]
    s0, s1 = streams

    with tc.tile_pool(name="psa", bufs=4, space="PSUM") as psa:
        # stream-staggered phase A: s0's AllReduce mesh (and its GN-scalar
        # chain on GpSimd) runs while s1's conv1 batches execute, so
        # phase B s0 can start right after s1's phase A PE work.
        s0.stage_a_init()
        for b0 in range(s0.NB):
            s0.stage_a_batch(psa, b0)
        s0.stage_a_finish(psa)          # AR1_s0 trigger
        s0.stage_b_start()
        s1.stage_a_init()
        for b0 in range(s1.NB):
            # s1 evacs alternate Scalar/DVE so s0's GN chunks (Scalar,
            # woven below) are not stuck behind a long Scalar queue
            s1.stage_a_batch(psa, b0, evac_dve=(b0 % 2 == 1))
            if b0 == 7:
                s0.post_ar1()           # DVE chain; AR1_s0 mesh done by now
            if b0 >= 9 and s0._gn_r < s0.RF:
                r1 = min(s0._gn_r + 8, s0.RF)
                s0._gn_chunk(s0._gn_r, r1)
                s0._gn_r = r1
        s1.stage_a_finish(psa)          # AR1_s1 trigger
    with tc.tile_pool(name="psb", bufs=2, space="PSUM") as psb:
        s1.stage_b_start()
        for t in range(s0.NST):
            s0.stage_b_tile(psb, t)
            if t == 2:
                s1.post_ar1()           # DVE chain; AR1_s1 mesh in flight
        # conv3 s0 starts immediately at the pair-pool drain point; the
        # AR2_s0 stats-pack (which also takes a pair-pool slot) goes after
        # two conv3 pairs -- its mesh has plenty of slack under B_s1
        s0.stage_c1_pair(psb, 0)
        s0.stage_c1_pair(psb, 1)
        s0.stage_b_finish(psb)          # AR2_s0 trigger
        for t2 in range(2, s0.NST // 2):
            s0.stage_c1_pair(psb, t2)   # conv3 s0 during AR2_s0 mesh
        for t in range(s1.NST):
            s1.stage_b_tile(psb, t)
            if t == 2:
                s0.post_ar2()
            if t >= 3 and t - 3 < s0.NST:
                s0.stage_c2_slice(t - 3)
        s1.stage_b_finish(psb)          # AR2_s1 trigger
        for t2 in range(s1.NST // 2):
            s1.stage_c1_pair(psb, t2)   # conv3 s1 during AR2_s1 mesh
        for t in range(s1.NST - 3, s0.NST):
            s0.stage_c2_slice(t)
        s1.post_ar2()
        for t2 in range(s1.NST // 2):
            s1.stage_c2_pair(t2, on_scalar=(t2 % 2 == 0))


def build_program(rows_slab=ROWS_SLAB, n_cores=N_CORES, n_samples=B,
                  n_streams=S):
    import contextlib
    cps = n_cores * n_streams // n_samples      # cores per sample
    groups = [list(range(a * cps, (a + 1) * cps))
              for a in range(n_cores // cps)]
    RF = rows_slab + 2
    nc = bacc.Bacc("TRN2", target_bir_lowering=False, debug=False,
                   enable_asserts=False, num_devices=n_cores)
    io = {
        "xs": nc.dram_tensor("xs", [n_streams, C, RF, W], F16,
                             kind="ExternalInput").ap(),
        "em": nc.dram_tensor("em", [1, 2 * n_streams], F32,
                             kind="ExternalInput").ap(),
        "w1t": nc.dram_tensor("w1t", [C, M], F16, kind="ExternalInput").ap(),
        "wbm": nc.dram_tensor("wbm", [C + 1, 6 * M], F16,
                              kind="ExternalInput").ap(),
        "w3g16": nc.dram_tensor("w3g16", [C, M], F16,
                                kind="ExternalInput").ap(),
        "cols": nc.dram_tensor("cols", [C, 7], F32, kind="ExternalInput").ap(),
        "onesrow": nc.dram_tensor("onesrow", [1, WP], F16,
                                  kind="ExternalInput").ap(),
        "out": nc.dram_tensor("out", [n_streams, C, rows_slab, W], F32,
                              kind="ExternalOutput").ap(),
    }
    with tile.TileContext(nc) as tc:
        with contextlib.ExitStack() as ctx:
            _emit(nc, tc, ctx, rows_slab, groups, io)
    nc.compile()
    return nc


def host_inputs(x, w1, b1, w21, b21, w22, b22, w3, b3,
                gn1_w, gn1_b, gn2_w, gn2_b,
                rows_slab=ROWS_SLAB, n_cores=N_CORES, n_streams=S):
    x = np.asarray(x, np.float32)
    nb_, _, hh, _ = x.shape
    cps = n_cores * n_streams // nb_
    w1 = np.asarray(w1, np.float32)
    w21 = np.asarray(w21, np.float32)
    w22 = np.asarray(w22, np.float32)
    w3 = np.asarray(w3, np.float32)

    w1t = np.zeros((C, M), np.float16)
    w1t[:, 0:C] = w1.T
    wbm = np.zeros((C + 1, 6 * M), np.float16)
    biases = (np.asarray(b21, np.float32), np.asarray(b22, np.float32))
    for wi, wmat in enumerate((w21, w22)):
        wt = np.ascontiguousarray(wmat.T).astype(np.float16)
        for j in range(3):
            blk = np.zeros((C + 1, M), np.float16)
            blk[32 * j:32 * j + 32, 0:C] = wt[32 * j:32 * j + 32, :]
            if j == 0:
                blk[C, 0:C] = biases[wi].astype(np.float16)
            wbm[:, (3 * wi + j) * M:(3 * wi + j + 1) * M] = blk
    w3g16 = np.zeros((C, M), np.float16)
    w3g16[:, 0:C] = (w3 * np.asarray(gn2_w)[None, :]).T.astype(np.float16)
    shared = {
        "w1t": w1t,
        "wbm": wbm,
        "onesrow": np.ones((1, WP), np.float16),
        "w3g16": w3g16,
        "cols": np.ascontiguousarray(np.stack(
            [np.asarray(b1, np.float32), np.asarray(gn1_w, np.float32),
             np.asarray(gn1_b, np.float32), np.asarray(b21, np.float32),
             np.asarray(b22, np.float32),
             (np.asarray(b3) + w3 @ np.asarray(gn2_b)).astype(np.float32),
             (w3 * np.asarray(gn2_w)[None, :]).sum(1).astype(np.float32)],
            axis=1)),
    }
    x16 = x.astype(np.float16)
    in_maps = []
    for k in range(n_cores):
        xs = np.zeros((n_streams, C, rows_slab + 2, W), np.float16)
        em = np.zeros((1, 2 * n_streams), np.float32)
        for s in range(n_streams):
            bidx = s * (nb_ // n_streams) + k // cps
            q = k % cps
            h0 = q * rows_slab
            lo, hi = h0 - 1, h0 + rows_slab + 1
            slo, shi = max(lo, 0), min(hi, hh)
            xs[s, :, slo - lo:slo - lo + (shi - slo), :] = \
                x16[bidx, :, slo:shi, :]
            em[0, 2 * s] = 1.0 if lo >= 0 else 0.0
            em[0, 2 * s + 1] = 1.0 if hi <= hh else 0.0
        in_maps.append({"xs": xs, "em": em, **shared})
    return in_maps


def gather_output(results, rows_slab=ROWS_SLAB, n_cores=N_CORES,
                  n_streams=S, n_samples=B, hh=H):
    cps = n_cores * n_streams // n_samples
    out = np.empty((n_samples, C, hh, W), np.float32)
    for k in range(n_cores):
        for s in range(n_streams):
            bidx = s * (n_samples // n_streams) + k // cps
            q = k % cps
            out[bidx, :, q * rows_slab:(q + 1) * rows_slab, :] = \
                results[k]["out"][s]
    return out


_PROGRAM = None


def kernel(x, w1, b1, w21, b21, w22, b22, w3, b3, gn1_w, gn1_b, gn2_w, gn2_b):
    global _PROGRAM
    from concourse.bass_utils import run_bass_kernel_spmd
    from concourse.bass_interp import get_hw_module
    if _PROGRAM is None:
        nc = build_program()
        nc.m = get_hw_module(nc.m)
        _PROGRAM = nc
    nc = _PROGRAM
    in_maps = host_inputs(x, w1, b1, w21, b21, w22, b22, w3, b3,
                          gn1_w, gn1_b, gn2_w, gn2_b)
    res = run_bass_kernel_spmd(nc, in_maps, core_ids=list(range(N_CORES)))
    return gather_output(res.results)


# revision 46
# speedup vs baseline: 1.1759x; 1.1759x over previous
"""Trainium2 Bass kernel for the AxialShift block (4x96x256x256, fp32).

Self-contained: builds an 8-core SPMD Bass program, compiles it once,
and runs it via run_bass_kernel_spmd.

Sharding: each core runs S=2 streams; stream s of core k handles a
quarter-sample slab (64 rows) of sample 2s + k//4.

Pipeline per core (x read once as fp16, out written once):
  phase A : conv1 as N=1024 fp16 matmuls over the 66-row frame (64 true
            rows + halo); PSUM evacuated fp16 into a zero-padded
            [96,66,258] frame on ScalarE; per-channel bn_stats on DVE.
  AR1     : ONE 16-byte AllReduce carrying both streams' GroupNorm-1
            partials (groups of 4 cores sharing a sample).
  GN+B    : GN1 scale/bias + erf-Gelu in place (ScalarE; rsqrt computed
            on DVE via bit-trick+Newton so no Sqrt ACT-table load); the
            4 axial-shift branch convs as 12 chunk-masked K=97 N=1024
            matmuls per 4-row supertile, grouped in branch pairs into
            [M,2,1024] PSUM tiles; one fused Gelu ACTIVATE per pair;
            branch sums split DVE/GpSimd; bn_stats for GroupNorm-2.
  AR2_s   : per-stream 8-byte AllReduce; conv3 (weights pre-folded with
            gamma2 on host, unscaled by std) runs immediately after each
            stream's phase B, writing fp16 results in place over the
            branch-sum buffer. Post-AR2 the per-sample 1/std2 scale and
            bias are applied on DVE/ScalarE and tiles stream out, so
            stream 0's stores overlap stream 1's compute.
"""
import sys

sys.path.insert(0, "/opt/trn_rl_repo")

import numpy as np

import concourse.bass as bass
import concourse.bacc as bacc
import concourse.tile as tile
from concourse import mybir

F32 = mybir.dt.float32
F16 = mybir.dt.float16
I32 = mybir.dt.int32

C = 96
M = 128           # matmul output width (96 channels + 32 zero pad -> FWL)
H = 256
W = 256
B = 4
WP = W + 2
N_CORES = 8
S = 2             # streams per core
ROWS_SLAB = H * B // (N_CORES * S)       # 64
EPS = 1e-5
AF = mybir.ActivationFunctionType
ALU = mybir.AluOpType
MAGIC = 0x5F3759DF

# (dh, dw) read offsets per chunk j=0,1,2 (s_j = -1, 0, +1):
BR_LR = [(0, 1), (0, 0), (0, -1)]
BR_LDIAG = [(1, 1), (0, 0), (-1, -1)]
BR_TD = [(1, 0), (0, 0), (-1, 0)]
BR_RDIAG = [(1, -1), (0, 0), (-1, 1)]
PAIRS = [(0, [BR_LR, BR_LDIAG]), (3, [BR_TD, BR_RDIAG])]


def _bcast(ap, nparts):
    return bass.AP(tensor=ap.tensor, offset=ap.offset,
                   ap=[[0, nparts]] + list(ap.ap[1:]))


def _rsqrt_cols(eng, con, ve, name):
    """rsqrt(ve) for a [C, ncol] fp32 tile on DVE or GpSimd (no ACT table).

    Quake initial guess + 3 Newton iterations (rel err ~4e-6)."""
    ncol = ve.shape[1]
    y = con.tile([C, ncol], F32, name=f"{name}_y")
    t = con.tile([C, ncol], F32, name=f"{name}_t")
    eng.tensor_scalar(out=t[:].bitcast(I32), in0=ve[:].bitcast(I32),
                      scalar1=1, scalar2=None,
                      op0=ALU.logical_shift_right)
    eng.tensor_scalar(out=y[:].bitcast(I32), in0=t[:].bitcast(I32),
                      scalar1=-1, scalar2=MAGIC,
                      op0=ALU.mult, op1=ALU.add)
    for _ in range(3):
        eng.tensor_mul(out=t[:], in0=y[:], in1=y[:])
        eng.tensor_mul(out=t[:], in0=t[:], in1=ve[:])
        eng.tensor_scalar(out=t[:], in0=t[:], scalar1=-0.5,
                          scalar2=1.5, op0=ALU.mult, op1=ALU.add)
        eng.tensor_mul(out=y[:], in0=y[:], in1=t[:])
    return y


class _Stream:
    """Per-stream state; stages are emitted by the orchestrator."""

    def __init__(self, nc, tc, pools, RH, groups, io, s):
        self.nc, self.tc, self.s = nc, tc, s
        self.p = pools
        self.RH = RH                      # 64 true rows
        self.RF = RH + 2                  # 66 frame rows
        self.NB = (self.RF + 3) // 4      # 17 phase-A batches (4 rows)
        self.NST = RH // 4                # 16 supertiles (4 rows x 256)
        self.groups = groups
        self.inv_n = 1.0 / (len(groups[0]) * C)
        self.io = io
        con = pools["consts"]
        big = pools["big"]
        self.xact = big.tile([C + 1, self.RF, WP], F16, name=f"xact{s}")
        self.opre = big.tile([C, self.NST, 4 * W], F16, name=f"opre{s}")
        self.stats1 = con.tile([C, 2 * self.NB - 1, 6], F32, name=f"st1_{s}")
        self.stats2 = con.tile([C, 2 * self.NST, 6], F32, name=f"st2_{s}")
        dram = pools["dram"]
        self.d1i = dram.tile([1, 2], F32, name=f"d1i{s}")
        self.d1o = dram.tile([1, 2], F32, name=f"d1o{s}")
        self.d2i = dram.tile([1, 2], F32, name=f"d2i{s}")
        self.d2o = dram.tile([1, 2], F32, name=f"d2o{s}")

    # ---------------- phase A ----------------
    def stage_a_init(self):
        nc = self.nc
        nc.vector.memset(self.xact[0:C, :, 0:1], 0.0)
        nc.vector.memset(self.xact[0:C, :, WP - 1:WP], 0.0)
        onesrow = self.io["onesrow"][:]
        nc.gpsimd.dma_start(
            out=self.xact[C:C + 1, :, :],
            in_=bass.AP(tensor=onesrow.tensor, offset=onesrow.offset,
                        ap=[[0, 1], [0, self.RF]] + list(onesrow.ap[1:])))
        self._xt, self._xt_base = None, 0

    def stage_a_batch(self, psa, b0, evac_dve=False):
        nc, s = self.nc, self.s
        xin = self.p["xin"]
        xs = self.io["xs"][s]
        r0 = 4 * b0
        nr = min(4, self.RF - r0)
        if r0 % 8 == 0:
            self._xt = xin.tile([C, 8, W], F16, tag="xt")
            nrows = min(8, self.RF - r0)
            nc.sync.dma_start(out=self._xt[:, 0:nrows, :],
                              in_=xs[:, r0:r0 + nrows, :])
            self._xt_base = r0
        xt = self._xt
        rr = r0 - self._xt_base
        pt = psa.tile([M, 4, W], F32, tag="pta")
        for h in range(0, nr, 2):
            nc.tensor.matmul(
                out=pt[:, h:h + 2, :].rearrange("p r w -> p (r w)"),
                lhsT=self.p["w1t"][:],
                rhs=xt[:, rr + h:rr + h + 2, :],
                start=True, stop=True)
        if evac_dve:
            nc.vector.tensor_copy(out=self.xact[0:C, r0:r0 + nr, 1:W + 1],
                                  in_=pt[0:C, 0:nr, :])
        else:
            nc.scalar.copy(out=self.xact[0:C, r0:r0 + nr, 1:W + 1],
                           in_=pt[0:C, 0:nr, :])
        if b0 == 0:
            nc.vector.bn_stats(out=self.stats1[:, 0, :],
                               in_=pt[0:C, 1:3, :].rearrange(
                                   "p r w -> p (r w)"))
            nc.vector.bn_stats(out=self.stats1[:, 1, :], in_=pt[0:C, 3, :])
        elif r0 + nr >= self.RF:
            nc.vector.bn_stats(out=self.stats1[:, 2 * b0, :],
                               in_=pt[0:C, 0, :])
        else:
            for h in range(2):
                nc.vector.bn_stats(
                    out=self.stats1[:, 2 * b0 + h, :],
                    in_=pt[0:C, 2 * h:2 * h + 2, :].rearrange(
                        "p r w -> p (r w)"))

    def stage_a_finish(self, psa):
        nc, s = self.nc, self.s
        con = self.p["consts"]
        cols = self.p["cols"]
        mv1 = con.tile([C, 2], F32, name=f"mv1_{s}")
        nc.vector.bn_aggr(out=mv1[:], in_=self.stats1[:])
        pack1 = con.tile([C, 2], F32, name=f"pk1_{s}")
        nc.vector.tensor_add(out=pack1[:, 0:1], in0=mv1[:, 0:1],
                             in1=cols[:, 0:1])
        tsq = con.tile([C, 1], F32, name=f"t1q_{s}")
        nc.vector.tensor_mul(out=tsq[:], in0=pack1[:, 0:1], in1=pack1[:, 0:1])
        nc.vector.tensor_add(out=pack1[:, 1:2], in0=mv1[:, 1:2], in1=tsq[:])
        spt = psa.tile([M, 4, W], F32, tag="pta")
        nc.tensor.matmul(out=spt[0:1, 0, 0:2], lhsT=self.p["ones96"][:],
                         rhs=pack1[:], start=True, stop=True)
        ar_in = con.tile([1, 2], F32, name=f"ar1i_{s}")
        nc.scalar.copy(out=ar_in[:], in_=spt[0:1, 0, 0:2])
        nc.sync.dma_start(out=self.d1i[:], in_=ar_in[:])
        nc.gpsimd.collective_compute(
            "AllReduce", ALU.add, replica_groups=self.groups,
            ins=[self.d1i.opt()], outs=[self.d1o.opt()])

    # ---------------- GN1 scalars (DVE; emit where the queue has slack) --
    def post_ar1(self):
        nc, s = self.nc, self.s
        eng = nc.vector
        con = self.p["consts"]
        cols = self.p["cols"]
        ar1 = con.tile([C, 2], F32, name=f"ar1_{s}")
        nc.gpsimd.dma_start(out=ar1[:], in_=_bcast(self.d1o[:], C))
        mu = con.tile([C, 1], F32, name=f"mu1_{s}")
        eng.tensor_scalar_mul(out=mu[:], in0=ar1[:, 0:1],
                              scalar1=self.inv_n)
        ve = con.tile([C, 1], F32, name=f"v1_{s}")
        eng.tensor_scalar(out=ve[:], in0=ar1[:, 1:2],
                          scalar1=self.inv_n, scalar2=EPS,
                          op0=ALU.mult, op1=ALU.add)
        musq = con.tile([C, 1], F32, name=f"mq1_{s}")
        eng.tensor_mul(out=musq[:], in0=mu[:], in1=mu[:])
        eng.tensor_sub(out=ve[:], in0=ve[:], in1=musq[:])
        inv = _rsqrt_cols(eng, con, ve, f"rs1_{s}")
        self.scale1 = con.tile([C, 1], F32, name=f"sc1_{s}")
        eng.tensor_mul(out=self.scale1[:], in0=inv[:], in1=cols[:, 1:2])
        self.bias1 = con.tile([C, 1], F32, name=f"bi1_{s}")
        eng.tensor_sub(out=self.bias1[:], in0=cols[:, 0:1], in1=mu[:])
        eng.tensor_mul(out=self.bias1[:], in0=self.bias1[:],
                       in1=self.scale1[:])
        eng.tensor_add(out=self.bias1[:], in0=self.bias1[:],
                       in1=cols[:, 2:3])

    def _gn_chunk(self, r0, r1):
        nc = self.nc
        nc.scalar.activation(out=self.xact[0:C, r0:r1, 1:W + 1],
                             in_=self.xact[0:C, r0:r1, 1:W + 1],
                             func=AF.Gelu, bias=self.bias1[:],
                             scale=self.scale1[:])
        if r0 == 0:
            nc.vector.tensor_scalar_mul(out=self.xact[0:C, 0:1, :],
                                        in0=self.xact[0:C, 0:1, :],
                                        scalar1=self.p["em"][:, 2 * self.s:
                                                            2 * self.s + 1])
        if r1 == self.RF:
            nc.vector.tensor_scalar_mul(
                out=self.xact[0:C, self.RF - 1:self.RF, :],
                in0=self.xact[0:C, self.RF - 1:self.RF, :],
                scalar1=self.p["em"][:, 2 * self.s + 1:2 * self.s + 2])

    # ---------------- GN-apply + phase B ----------------
    def stage_b_start(self):
        self._gn_r = 0

    def stage_b_tile(self, psb, st):
        nc, s = self.nc, self.s
        wbm = self.p["wbm"]
        gst, tmp = self.p["gst"], self.p["tmp"]
        need = min(4 * st + 6, self.RF)
        while self._gn_r < need:
            r1 = min(self._gn_r + 8, self.RF)
            self._gn_chunk(self._gn_r, r1)
            self._gn_r = r1
        pr = 4 * st + 1
        adds = []
        for pi, (wsel, branches) in enumerate(PAIRS):
            pt = psb.tile([M, 2, 4 * W], F32, tag="pair")
            for b, ds in enumerate(branches):
                for h in range(2):
                    for j, (dh, dw) in enumerate(ds):
                        bi = wsel + j
                        nc.tensor.matmul(
                            out=pt[:, b, 512 * h:512 * (h + 1)],
                            lhsT=wbm[:, bi * M:(bi + 1) * M],
                            rhs=self.xact[0:C + 1,
                                          pr + dh + 2 * h:pr + dh + 2 * h + 2,
                                          1 + dw:1 + dw + W],
                            start=(j == 0), stop=(j == 2))
            g = gst.tile([C, 2, 4 * W], F16, tag="g")
            nc.scalar.activation(out=g[:], in_=pt[0:C, :, :],
                                 func=AF.Gelu, bias=self.p["zcol"][:])
            a = tmp.tile([C, 4 * W], F16, tag=f"a{pi}")
            eng = nc.vector if pi == 0 else nc.gpsimd
            eng.tensor_add(out=a[:], in0=g[:, 0, :], in1=g[:, 1, :])
            adds.append(a)
        od = self.opre[:, st, :]
        nc.vector.tensor_add(out=od, in0=adds[0][:], in1=adds[1][:])
        for h in range(2):
            nc.vector.bn_stats(out=self.stats2[:, 2 * st + h, :],
                               in_=self.opre[:, st, 512 * h:512 * (h + 1)])

    def stage_b_finish(self, psb):
        nc, s = self.nc, self.s
        con = self.p["consts"]
        mv2 = con.tile([C, 2], F32, name=f"mv2_{s}")
        nc.vector.bn_aggr(out=mv2[:], in_=self.stats2[:])
        pack2 = con.tile([C, 2], F32, name=f"pk2_{s}")
        nc.vector.tensor_copy(out=pack2[:, 0:1], in_=mv2[:, 0:1])
        tsq = con.tile([C, 1], F32, name=f"t2q_{s}")
        nc.vector.tensor_mul(out=tsq[:], in0=mv2[:, 0:1], in1=mv2[:, 0:1])
        nc.vector.tensor_add(out=pack2[:, 1:2], in0=mv2[:, 1:2], in1=tsq[:])
        spt = psb.tile([M, 2, 4 * W], F32, tag="pair")
        nc.tensor.matmul(out=spt[0:1, 0, 0:2], lhsT=self.p["ones96"][:],
                         rhs=pack2[:], start=True, stop=True)
        ar_in = con.tile([1, 2], F32, name=f"ar2i_{s}")
        nc.scalar.copy(out=ar_in[:], in_=spt[0:1, 0, 0:2])
        nc.sync.dma_start(out=self.d2i[:], in_=ar_in[:])
        nc.gpsimd.collective_compute(
            "AllReduce", ALU.add, replica_groups=self.groups,
            ins=[self.d2i.opt()], outs=[self.d2o.opt()])

    # ---------------- conv3 (pre-AR2 scale), in place ----------------
    def stage_c1_pair(self, psb, t2):
        nc = self.nc
        w3g = self.p["w3g16"]
        pc = psb.tile([M, 2, 4 * W], F32, tag="pair")
        for j in range(2):
            t = 2 * t2 + j
            for h in range(2):
                nc.tensor.matmul(out=pc[:, j, 512 * h:512 * (h + 1)],
                                 lhsT=w3g[:],
                                 rhs=self.opre[:, t,
                                               512 * h:512 * (h + 1)],
                                 start=True, stop=True)
        # split the PSUM evacuation across engines so the pair-pool slot
        # frees quickly (a single 2048-elem DVE cast is a 2.3us hold)
        nc.scalar.copy(out=self.opre[:, 2 * t2, :], in_=pc[0:C, 0, :])
        nc.vector.tensor_copy(out=self.opre[:, 2 * t2 + 1, :],
                              in_=pc[0:C, 1, :])

    # ---------------- GN2 scalars (all DVE) ----------------
    def post_ar2(self):
        nc, s = self.nc, self.s
        con = self.p["consts"]
        cols = self.p["cols"]
        ar2 = con.tile([C, 2], F32, name=f"ar2_{s}")
        nc.sync.dma_start(out=ar2[:], in_=_bcast(self.d2o[:], C))
        mu = con.tile([C, 1], F32, name=f"mu2_{s}")
        nc.vector.tensor_scalar_mul(out=mu[:], in0=ar2[:, 0:1],
                                    scalar1=self.inv_n)
        ve = con.tile([C, 1], F32, name=f"v2_{s}")
        nc.vector.tensor_scalar(out=ve[:], in0=ar2[:, 1:2],
                                scalar1=self.inv_n, scalar2=EPS,
                                op0=ALU.mult, op1=ALU.add)
        musq = con.tile([C, 1], F32, name=f"mq2_{s}")
        nc.vector.tensor_mul(out=musq[:], in0=mu[:], in1=mu[:])
        nc.vector.tensor_sub(out=ve[:], in0=ve[:], in1=musq[:])
        self.inv2 = _rsqrt_cols(nc.vector, con, ve, f"rs2_{s}")
        self.ccol = con.tile([C, 1], F32, name=f"cc_{s}")
        nc.vector.tensor_mul(out=self.ccol[:], in0=self.inv2[:], in1=mu[:])
        nc.vector.tensor_mul(out=self.ccol[:], in0=self.ccol[:],
                             in1=cols[:, 6:7])
        nc.vector.tensor_sub(out=self.ccol[:], in0=cols[:, 5:6],
                             in1=self.ccol[:])

    # ---------------- post-AR2 scale + store ----------------
    def stage_c2_slice(self, t, on_scalar=False):
        nc = self.nc
        ost = self.p["ost"]
        out = self.io["out"][self.s]
        o = ost.tile([C, 4 * W], F32, tag="o")
        if on_scalar:
            nc.scalar.activation(out=o[:], in_=self.opre[:, t, :],
                                 func=AF.Identity, bias=self.ccol[:],
                                 scale=self.inv2[:])
        else:
            nc.vector.tensor_scalar(out=o[:], in0=self.opre[:, t, :],
                                    scalar1=self.inv2[:],
                                    scalar2=self.ccol[:],
                                    op0=ALU.mult, op1=ALU.add)
        nc.sync.dma_start(
            out=out[:, 4 * t:4 * t + 4, :],
            in_=o[:].rearrange("p (r w) -> p r w", w=W))

    def stage_c2_pair(self, t2, on_scalar=False):
        """Tail variant: scale+store two contiguous slices in one op and
        one store DMA (halves instruction overhead in the tail)."""
        nc = self.nc
        ost = self.p["ost"]
        out = self.io["out"][self.s]
        o = ost.tile([C, 2, 4 * W], F32, tag="o2")
        src = self.opre[:, 2 * t2:2 * t2 + 2, :]
        if on_scalar:
            nc.scalar.activation(out=o[:], in_=src,
                                 func=AF.Identity, bias=self.ccol[:],
                                 scale=self.inv2[:])
        else:
            nc.vector.tensor_scalar(out=o[:], in0=src,
                                    scalar1=self.inv2[:],
                                    scalar2=self.ccol[:],
                                    op0=ALU.mult, op1=ALU.add)
        nc.sync.dma_start(
            out=out[:, 8 * t2:8 * t2 + 8, :],
            in_=o[:].rearrange("p n (r w) -> p (n r) w", w=W))


def _emit(nc, tc, ctx, RH, groups, io):
    pools = {
        "consts": ctx.enter_context(tc.tile_pool(name="consts", bufs=1)),
        "big": ctx.enter_context(tc.tile_pool(name="big", bufs=1)),
        "xin": ctx.enter_context(tc.tile_pool(name="xin", bufs=3)),
        "gst": ctx.enter_context(tc.tile_pool(name="gst", bufs=3)),
        "tmp": ctx.enter_context(tc.tile_pool(name="tmp", bufs=4)),
        "ost": ctx.enter_context(tc.tile_pool(name="ost", bufs=2)),
        "dram": ctx.enter_context(tc.tile_pool(name="dram", bufs=1,
                                               space="DRAM")),
    }
    con = pools["consts"]
    # consts go on the vector/scalar DMA queues so the sync queue starts
    # with the first xs input loads (phase A ramps ~5us earlier)
    w1t = con.tile([C, M], F16)
    nc.scalar.dma_start(out=w1t[:], in_=io["w1t"][:])
    wbm = con.tile([C + 1, 6 * M], F16)
    nc.scalar.dma_start(out=wbm[:], in_=io["wbm"][:])
    w3g16 = con.tile([C, M], F16)
    nc.scalar.dma_start(out=w3g16[:], in_=io["w3g16"][:])
    cols = con.tile([C, 7], F32)
    nc.scalar.dma_start(out=cols[:], in_=io["cols"][:])
    em = con.tile([C, 2 * S], F32)
    nc.gpsimd.dma_start(out=em[:], in_=_bcast(io["em"][:], C))
    ones96 = con.tile([C, 1], F32)
    nc.vector.memset(ones96[:], 1.0)
    zcol = con.tile([C, 1], F32)
    nc.vector.memset(zcol[:], 0.0)
    # preload the Gelu ACT table so the first GN chunk (on the critical
    # path right after AR1) doesn't pay the ~2.6us ACT_TABLE_LOAD
    gwarm = con.tile([C, 1], F32)
    nc.scalar.activation(out=gwarm[:], in_=zcol[:], func=AF.Gelu,
                         bias=zcol[:])
    pools.update(w1t=w1t, wbm=wbm, w3g16=w3g16, cols=cols, em=em,
                 ones96=ones96, zcol=zcol)

    dram = pools["dram"]
    # warm up the collectives firmware path (result unused)
    dw_i = dram.tile([1, 2], F32)
    dw_o = dram.tile([1, 2], F32)
    warm = con.tile([1, 2], F32)
    nc.vector.memset(warm[:], 0.0)
    nc.sync.dma_start(out=dw_i[:], in_=warm[:])
    nc.gpsimd.collective_compute("AllReduce", ALU.add, replica_groups=groups,
                                 ins=[dw_i.opt()], outs=[dw_o.opt()])

    streams = [_Stream(nc, tc, pools, RH, groups, io, s) for s in range(S)]
    s0, s1 = streams

    with tc.tile_pool(name="psa", bufs=4, space="PSUM") as psa:
        # stream-staggered phase A: s0's AllReduce mesh (and its GN-scalar
        # chain on GpSimd) runs while s1's conv1 batches execute, so
        # phase B s0 can start right after s1's phase A PE work.
        s0.stage_a_init()
        for b0 in range(s0.NB):
            s0.stage_a_batch(psa, b0)
        s0.stage_a_finish(psa)          # AR1_s0 trigger
        s0.stage_b_start()
        s1.stage_a_init()
        for b0 in range(s1.NB):
            # s1 evacs alternate Scalar/DVE so s0's GN chunks (Scalar,
            # woven below) are not stuck behind a long Scalar queue
            s1.stage_a_batch(psa, b0, evac_dve=(b0 % 2 == 1))
            if b0 == 7:
                s0.post_ar1()           # DVE chain; AR1_s0 mesh done by now
            if b0 >= 9 and (b0 - 9) % 3 == 0 and s0._gn_r < s0.RF:
                r1 = min(s0._gn_r + 8, s0.RF)
                s0._gn_chunk(s0._gn_r, r1)
                s0._gn_r = r1
        s1.stage_a_finish(psa)          # AR1_s1 trigger
    with tc.tile_pool(name="psb", bufs=2, space="PSUM") as psb:
        s1.stage_b_start()
        for t in range(s0.NST):
            s0.stage_b_tile(psb, t)
            if t == 2:
                s1.post_ar1()           # DVE chain; AR1_s1 mesh in flight
        # conv3 s0 starts immediately at the pair-pool drain point; the
        # AR2_s0 stats-pack (which also takes a pair-pool slot) goes after
        # two conv3 pairs -- its mesh has plenty of slack under B_s1
        s0.stage_c1_pair(psb, 0)
        s0.stage_c1_pair(psb, 1)
        s0.stage_b_finish(psb)          # AR2_s0 trigger
        for t2 in range(2, s0.NST // 2):
            s0.stage_c1_pair(psb, t2)   # conv3 s0 during AR2_s0 mesh
        for t in range(s1.NST):
            s1.stage_b_tile(psb, t)
            if t == 2:
                s0.post_ar2()
            if t >= 3 and t - 3 < s0.NST:
                s0.stage_c2_slice(t - 3)
        s1.stage_b_finish(psb)          # AR2_s1 trigger
        for t2 in range(s1.NST // 2):
            s1.stage_c1_pair(psb, t2)   # conv3 s1 during AR2_s1 mesh
        for t in range(s1.NST - 3, s0.NST):
            s0.stage_c2_slice(t)
        s1.post_ar2()
        for t2 in range(s1.NST // 2):
            s1.stage_c2_pair(t2, on_scalar=(t2 % 2 == 0))


def build_program(rows_slab=ROWS_SLAB, n_cores=N_CORES, n_samples=B,
                  n_streams=S):
    import contextlib
    cps = n_cores * n_streams // n_samples      # cores per sample
    groups = [list(range(a * cps, (a + 1) * cps))
              for a in range(n_cores // cps)]
    RF = rows_slab + 2
    nc = bacc.Bacc("TRN2", target_bir_lowering=False, debug=False,
                   enable_asserts=False, num_devices=n_cores)
    io = {
        "xs": nc.dram_tensor("xs", [n_streams, C, RF, W], F16,
                             kind="ExternalInput").ap(),
        "em": nc.dram_tensor("em", [1, 2 * n_streams], F32,
                             kind="ExternalInput").ap(),
        "w1t": nc.dram_tensor("w1t", [C, M], F16, kind="ExternalInput").ap(),
        "wbm": nc.dram_tensor("wbm", [C + 1, 6 * M], F16,
                              kind="ExternalInput").ap(),
        "w3g16": nc.dram_tensor("w3g16", [C, M], F16,
                                kind="ExternalInput").ap(),
        "cols": nc.dram_tensor("cols", [C, 7], F32, kind="ExternalInput").ap(),
        "onesrow": nc.dram_tensor("onesrow", [1, WP], F16,
                                  kind="ExternalInput").ap(),
        "out": nc.dram_tensor("out", [n_streams, C, rows_slab, W], F32,
                              kind="ExternalOutput").ap(),
    }
    with tile.TileContext(nc) as tc:
        with contextlib.ExitStack() as ctx:
            _emit(nc, tc, ctx, rows_slab, groups, io)
    nc.compile()
    return nc


def host_inputs(x, w1, b1, w21, b21, w22, b22, w3, b3,
                gn1_w, gn1_b, gn2_w, gn2_b,
                rows_slab=ROWS_SLAB, n_cores=N_CORES, n_streams=S):
    x = np.asarray(x, np.float32)
    nb_, _, hh, _ = x.shape
    cps = n_cores * n_streams // nb_
    w1 = np.asarray(w1, np.float32)
    w21 = np.asarray(w21, np.float32)
    w22 = np.asarray(w22, np.float32)
    w3 = np.asarray(w3, np.float32)

    w1t = np.zeros((C, M), np.float16)
    w1t[:, 0:C] = w1.T
    wbm = np.zeros((C + 1, 6 * M), np.float16)
    biases = (np.asarray(b21, np.float32), np.asarray(b22, np.float32))
    for wi, wmat in enumerate((w21, w22)):
        wt = np.ascontiguousarray(wmat.T).astype(np.float16)
        for j in range(3):
            blk = np.zeros((C + 1, M), np.float16)
            blk[32 * j:32 * j + 32, 0:C] = wt[32 * j:32 * j + 32, :]
            if j == 0:
                blk[C, 0:C] = biases[wi].astype(np.float16)
            wbm[:, (3 * wi + j) * M:(3 * wi + j + 1) * M] = blk
    w3g16 = np.zeros((C, M), np.float16)
    w3g16[:, 0:C] = (w3 * np.asarray(gn2_w)[None, :]).T.astype(np.float16)
    shared = {
        "w1t": w1t,
        "wbm": wbm,
        "onesrow": np.ones((1, WP), np.float16),
        "w3g16": w3g16,
        "cols": np.ascontiguousarray(np.stack(
            [np.asarray(b1, np.float32), np.asarray(gn1_w, np.float32),
             np.asarray(gn1_b, np.float32), np.asarray(b21, np.float32),
             np.asarray(b22, np.float32),
             (np.asarray(b3) + w3 @ np.asarray(gn2_b)).astype(np.float32),
             (w3 * np.asarray(gn2_w)[None, :]).sum(1).astype(np.float32)],
            axis=1)),
    }
    x16 = x.astype(np.float16)
    in_maps = []
    for k in range(n_cores):
        xs = np.zeros((n_streams, C, rows_slab + 2, W), np.float16)
        em = np.zeros((1, 2 * n_streams), np.float32)
        for s in range(n_streams):
            bidx = s * (nb_ // n_streams) + k // cps
            q = k % cps
            h0 = q * rows_slab
            lo, hi = h0 - 1, h0 + rows_slab + 1
            slo, shi = max(lo, 0), min(hi, hh)
            xs[s, :, slo - lo:slo - lo + (shi - slo), :] = \
                x16[bidx, :, slo:shi, :]
            em[0, 2 * s] = 1.0 if lo >= 0 else 0.0
            em[0, 2 * s + 1] = 1.0 if hi <= hh else 0.0
        in_maps.append({"xs": xs, "em": em, **shared})
    return in_maps


def gather_output(results, rows_slab=ROWS_SLAB, n_cores=N_CORES,
                  n_streams=S, n_samples=B, hh=H):
    cps = n_cores * n_streams // n_samples
    out = np.empty((n_samples, C, hh, W), np.float32)
    for k in range(n_cores):
        for s in range(n_streams):
            bidx = s * (n_samples // n_streams) + k // cps
            q = k % cps
            out[bidx, :, q * rows_slab:(q + 1) * rows_slab, :] = \
                results[k]["out"][s]
    return out


_PROGRAM = None


def kernel(x, w1, b1, w21, b21, w22, b22, w3, b3, gn1_w, gn1_b, gn2_w, gn2_b):
    global _PROGRAM
    from concourse.bass_utils import run_bass_kernel_spmd
    from concourse.bass_interp import get_hw_module
    if _PROGRAM is None:
        nc = build_program()
        nc.m = get_hw_module(nc.m)
        _PROGRAM = nc
    nc = _PROGRAM
    in_maps = host_inputs(x, w1, b1, w21, b21, w22, b22, w3, b3,
                          gn1_w, gn1_b, gn2_w, gn2_b)
    res = run_bass_kernel_spmd(nc, in_maps, core_ids=list(range(N_CORES)))
    return gather_output(res.results)
